# revision 41
# baseline (speedup 1.0000x reference)
"""Trainium2 Bass kernel for nn_MCQuantiles (ThreeCompNode SNN scan).

Strategy (8 NeuronCores, data-parallel over batch):
- Each core takes 8 batches x 32 samples = 256 rows of the B*S axis.
- Everything runs in "transposed space": feature dims on SBUF partitions,
  batch-rows on the free dim. All transposes/swizzles/casts are host-side;
  every DMA is a flat contiguous [128, X] block.
- All matmuls run in fp8(e4m3) DoubleRow mode: 2 k-tiles per PE pass = 2x
  bf16 throughput.
- KEY RESTRUCTURE vs v1: G_t = 2^t(ma_t+mb_t) = sum_{s<=t} 2^{s-1}(ap_s+ba_s)
  is LINEAR in the embeddings, so the time-cumsum is folded into te/se on the
  HOST (tec_t = cumsum of 2^{t-1}*te*EMB_SC). One matmul then yields G_t
  directly -- no G recurrence on device:
      P_t  = ap_psum_t * c + bexpcum_t          (bexpcum = cumulative basal)
      M_t  = P_t + Mmask_{t-1},  q = (M <= 2^{t+1}),  Mmask = q*M (gpsimd)
- Layer-1 feeds W1 with the spike sp (not NOT-spike): with b1 == 0 there is
  then NO derived constant, so the hq psum PRELOAD is a single exact Act
  Copy (-W_SC/2^t)*mlmask_{t-1} (the table-based Identity+bias path clips
  large inputs on hardware -- avoid it). W1 wall is negated host-side so the
  one-scale eviction ml_t = psum * (-2^t/W_SC) = mlmask + 2^t*(W1@sp) comes
  out with the right sign. The whole ML update runs on the (otherwise idle)
  Act engine; DVE does the compares + ml mask. (A nonzero b1 would use one
  extra constant k-pair, baseline-style.)
- Layer-2 feeds sp2 directly into W2 (fp8): zero spikes accumulate an
  exactly-zero psum, preserving the bit-exact b2 output.
- ap psum is 4 per-g single-bank tiles; the per-g stt eviction releases each
  bank individually so the next pair's g-matmuls restart sooner.
- Startup DMA triggers are spread across engine queues (sync/scalar/vector/
  gpsimd) instead of serializing ~10 x 640ns on the Sync queue.
"""
import numpy as np
import ml_dtypes

import bass_rust
import concourse.bass as bass
import concourse.mybir as mybir
from concourse.bass_utils import run_bass_kernel_spmd
from concourse.tile import TileContext
from concourse.tile_rust import add_dep_helper

# ----- problem constants (hardcoded per contract) -----
T, B, S = 8, 64, 32
DS = DT = 3136
F = H = 512
L = 18
N_CORES = 8
NB = B // N_CORES              # 8 batches per core
R = NB * S                     # 256 rows per core
KD = 3328                      # 3136 padded to 26 k-tiles of 128
NK2 = KD // 256                # 13 DoubleRow k-pairs
NPAIR = T // 2                 # 4 step pairs
NG = F // 128                  # 4 f-tiles (= h-tiles)
NH2 = 2                        # W1 contraction: 2 DR pairs (c1 via psum preload)

WA_COLS = NK2 * 2 * F          # fp8 apical weight wall
O_WB = 0                       # wallM: basal weights
O_SE = NK2 * 2 * F             # then state embeddings
WM_COLS = O_SE + NK2 * 2 * T * NB
WB_COLS = NH2 * 2 * H          # fp8 W1 wall (2 DR pairs only)
LP = 32                        # W2 k-tile column pitch (L=18 padded for align)
W2_COLS = 2 * 2 * LP           # fp8 W2 wall, 2 DR pairs

# scales folded host-side (see prep_in_maps)
EMB_SC = 0.25                  # global embedding scale (te/se * 2^{t-1} * EMB_SC)
W_SC = 512.0                   # weight scale for Wa/Wb/W1/W2
PSUM_DESC = 1.0 / (EMB_SC * W_SC)   # apical/basal psum -> G increment

F32 = mybir.dt.float32
BF16 = mybir.dt.bfloat16
FP8 = mybir.dt.float8e4
OP = mybir.AluOpType
DRMODE = mybir.MatmulPerfMode.DoubleRow
ACT_COPY = mybir.ActivationFunctionType.Copy
ACT_ID = mybir.ActivationFunctionType.Identity

# te DMA chunking in DR k-pair units (13 total); first small for startup
CHUNKS = [1, 2, 2, 4, 4]
CH_OFF = [0, 1, 3, 5, 9]
NCHUNK = len(CHUNKS)


def _patch_tile_drain():
    """This walrus build allows a single sync-wait per TPB_CTRL Drain; Tile's
    kernel-tail drain attaches one wait per active logical proc. Split them
    across a chain of drains."""
    def _patched(self, tick_clock, wait_clock):
        nc = self.nc
        drain_inst = nc.sync.drain()
        wait_clock.add_sem_waits(
            drain_inst.ins, bass_rust.ScopedClock({None: tick_clock.global_clock})
        )
        si = drain_inst.ins.sync_info
        if si is not None and len(si.on_wait) > 1:
            waits = list(si.on_wait)
            drain_inst.ins.sync_info = mybir.SyncInfo(
                on_wait=waits[:1], on_update=list(si.on_update)
            )
            for w in waits[1:]:
                extra = nc.sync.drain()
                extra.ins.sync_info = mybir.SyncInfo(on_wait=[w], on_update=[])
        nc.all_engine_barrier()
        popped = nc._tile_sem_poison_stack.pop()
        assert popped is self._sem_poison
        nc.clear_and_free_semaphores(list(self.sems.allocated().values()))
        nc.all_engine_barrier()

    TileContext._drain_and_barrier = _patched


def _split_excess_waits(nc, limit=1):
    """Walrus here rejects instructions carrying more than ~1 sync-wait. Move
    excess waits onto same-engine NoOps inserted just before the instruction."""
    for fn in nc.m.functions:
        for bb in fn.blocks:
            new = []
            changed = False
            for inst in bb.instructions:
                si = getattr(inst, "sync_info", None)
                ow = list(si.on_wait) if si is not None and si.on_wait else []
                if len(ow) > limit:
                    extra = ow[limit:]
                    for j in range(0, len(extra), limit):
                        nop = mybir.InstNoOp(
                            name=f"{inst.name}-ws{j}", ins=[], outs=[]
                        )
                        nop.engine = inst.engine
                        nop.sync_info = mybir.SyncInfo(
                            on_wait=extra[j : j + limit], on_update=[]
                        )
                        new.append(nop)
                    inst.sync_info = mybir.SyncInfo(
                        on_wait=ow[:limit], on_update=list(si.on_update)
                    )
                    changed = True
                new.append(inst)
            if changed:
                try:
                    bb.instructions[:] = new
                except TypeError:
                    bb.instructions = new


def build_nc():
    _patch_tile_drain()
    nc = bass.Bass()

    teT = nc.declare_dram_parameter("teT", [NPAIR, 128, NK2 * 2 * 2 * R], FP8,
                                    isOutput=False)
    wallA = nc.declare_dram_parameter("wallA", [128, WA_COLS], FP8, isOutput=False)
    wallM = nc.declare_dram_parameter("wallM", [128, WM_COLS], FP8, isOutput=False)
    wallB = nc.declare_dram_parameter("wallB", [128, WB_COLS], FP8, isOutput=False)
    w2w = nc.declare_dram_parameter("w2w", [128, W2_COLS], FP8, isOutput=False)
    cons = nc.declare_dram_parameter("cons", [128, 1], F32, isOutput=False)
    out = nc.declare_dram_parameter("out", [L, R], F32, isOutput=True)

    with TileContext(nc) as tc:
        with (
            tc.tile_pool(name="wpool", bufs=1) as wpool,
            tc.tile_pool(name="tepool", bufs=2) as tepool,
            tc.tile_pool(name="state", bufs=1) as state,
            tc.tile_pool(name="qpool", bufs=3) as qpool,
            tc.tile_pool(name="gpool", bufs=2) as gpool,
            tc.tile_pool(name="appool", bufs=1, space="PSUM") as appool,
            tc.tile_pool(name="hpool", bufs=1, space="PSUM") as hpool,
            tc.tile_pool(name="bpool", bufs=1, space="PSUM") as bpool,
            tc.tile_pool(name="opool", bufs=1, space="PSUM") as opool,
        ):
            # ---- startup DMAs: spread across engine queues so descriptor
            # generation parallelizes; order within each queue by need-time.
            wallM_sb = wpool.tile([128, WM_COLS], FP8, tag="wallM", name="wallM_sb")
            wallB_sb = wpool.tile([128, WB_COLS], FP8, tag="wallB", name="wallB_sb")
            w2_sb = wpool.tile([128, W2_COLS], FP8, tag="w2w", name="w2_sb")
            cons_sb = wpool.tile([128, 1], F32, tag="cons", name="cons_sb")

            def te_dma(tck, pair, c):
                nc.sync.dma_start(
                    tck[:],
                    teT[pair][:, CH_OFF[c] * 4 * R
                              : (CH_OFF[c] + CHUNKS[c]) * 4 * R],
                )

            te0_tiles = []
            wallA_c = []
            for c in range(NCHUNK):
                tck = tepool.tile(
                    [128, CHUNKS[c] * 2 * 2 * R], FP8, tag=f"te{c}",
                    name=f"te_ck{c}",
                )
                te0_tiles.append(tck)
                te_dma(tck, 0, c)
                wa_ck = wpool.tile(
                    [128, CHUNKS[c] * 2 * F], FP8, tag=f"wallA{c}", name=f"wa_ck{c}"
                )
                wallA_c.append(wa_ck)
                nc.scalar.dma_start(
                    wa_ck[:],
                    wallA[:, CH_OFF[c] * 2 * F : (CH_OFF[c] + CHUNKS[c]) * 2 * F],
                )
            nc.gpsimd.dma_start(wallM_sb[:], wallM[:])
            nc.gpsimd.dma_start(wallB_sb[:], wallB[:])
            nc.gpsimd.dma_start(w2_sb[:], w2w[:])
            nc.gpsimd.dma_start(cons_sb[:], cons[:])

            def waT(kk, g):
                # lhsT [128, 2, 128] for DR pair kk, out tile g
                for c in range(NCHUNK - 1, -1, -1):
                    if kk >= CH_OFF[c]:
                        k = kk - CH_OFF[c]
                        v = wallA_c[c][:, k * 2 * F : (k + 1) * 2 * F].rearrange(
                            "p (two f) -> p two f", two=2
                        )
                        return v[:, :, g * 128 : (g + 1) * 128]

            def wbT(kk, g):
                v = wallM_sb[:, O_WB + kk * 2 * F : O_WB + (kk + 1) * 2 * F]
                v = v.rearrange("p (two f) -> p two f", two=2)
                return v[:, :, g * 128 : (g + 1) * 128]

            def seT(kk):
                v = wallM_sb[:, O_SE + kk * 2 * T * NB : O_SE + (kk + 1) * 2 * T * NB]
                return v.rearrange("p (two n) -> p two n", two=2)

            def w1T(kk, g):
                v = wallB_sb[:, kk * 2 * H : (kk + 1) * 2 * H].rearrange(
                    "p (two h) -> p two h", two=2
                )
                return v[:, :, g * 128 : (g + 1) * 128]

            def w2T(kk):
                v = w2_sb[:, kk * 2 * LP : (kk + 1) * 2 * LP]
                v = v.rearrange("p (two l) -> p two l", two=2)
                return v[:, :, 0:L]

            evb_ap = cons_sb[0:L, 0:1]

            # ---- state tiles ----
            M = state.tile([128, NG * R], BF16, tag="M", name="M")
            Mm = state.tile([128, NG * R], BF16, tag="Mm", name="Mm")
            # Y = masked layer-1 psum carry: Y_t = psum_t * (psum_t >= -W_SC);
            # psum_{t+1} = 0.5*Y_t + W_SC*(-W1)@sp  (thresholds double per t).
            # Split in h-halves so each half's preload->W1->compare chain
            # pipelines independently (tile-granular deps).
            Yh = [state.tile([128, 2 * R], BF16, tag=f"Y{h}", name=f"Y{h}")
                  for h in range(2)]
            binc = state.tile([128, NG * T * NB], BF16, tag="binc", name="binc")

            # ap psum: one single-bank tile per g so evictions release banks
            # individually
            ap_ps = [appool.tile([128, 2 * R], F32, tag=f"ap{g}", name=f"ap{g}")
                     for g in range(NG)]
            o_psum = opool.tile([L, R], F32, tag="o", name="o_psum")

            nc.vector.memset(Mm[:], 0.0)
            nc.vector.memset(Yh[0][:], 0.0)
            nc.vector.memset(Yh[1][:], 0.0)

            # ---- te DMA + apical matmul emission ----
            def emit_te_dma(pair):
                tiles = []
                for c in range(NCHUNK):
                    tck = tepool.tile(
                        [128, CHUNKS[c] * 2 * 2 * R], FP8, tag=f"te{c}",
                        name=f"te_ck{c}",
                    )
                    tiles.append(tck)
                    te_dma(tck, pair, c)
                return tiles

            def emit_ap_chunk(te_tiles, c):
                for g in range(NG):
                    for k in range(CHUNKS[c]):
                        kk = CH_OFF[c] + k
                        rhs = te_tiles[c][:, k * 4 * R : (k + 1) * 4 * R].rearrange(
                            "p (two n) -> p two n", two=2
                        )
                        nc.tensor.matmul(
                            ap_ps[g][:],
                            lhsT=waT(kk, g),
                            rhs=rhs,
                            start=(kk == 0),
                            stop=(kk == NK2 - 1),
                            perf_mode=DRMODE,
                        )

            def emit_basal():
                bs_psum = bpool.tile([128, NG * T * NB], F32, tag="bs",
                                     name="bs_psum")
                for g in range(NG):
                    for kk in range(NK2):
                        nc.tensor.matmul(
                            bs_psum[:, g * T * NB : (g + 1) * T * NB],
                            lhsT=wbT(kk, g),
                            rhs=seT(kk),
                            start=(kk == 0),
                            stop=(kk == NK2 - 1),
                            perf_mode=DRMODE,
                        )
                # binc (g-major) = cumulative basal contribution to G
                nc.scalar.activation(binc[:], bs_psum[:], ACT_COPY,
                                     scale=PSUM_DESC)

            def binc_bc(t):
                v = binc[:].rearrange("p (g x) -> p g x", g=NG)
                v = v[:, :, t * NB : (t + 1) * NB]
                return v.unsqueeze(3).broadcast_to([128, NG, NB, S])

            def emit_bexp(bx, sub, t):
                # cumulative-basal expansion for timestep t into pair tile
                # half; bx layout is (sub, g, r)
                nc.scalar.activation(
                    bx[:, sub * NG * R : (sub + 1) * NG * R].rearrange(
                        "p (g b s) -> p g b s", g=NG, s=S),
                    binc_bc(t), ACT_COPY,
                )

            # ---- per-pair state math ----
            def emit_P(t0):
                # pair-0 only: P' = ap_psum * c (no bexp dependency -- basal
                # is still streaming in); PB adds bexp separately below.
                pt = gpool.tile([128, NG * 2 * R], BF16, tag="P", name=f"P{t0}")
                ptv = pt[:].rearrange("p (two gr) -> p two gr", two=2)
                for g in range(NG):
                    nc.vector.tensor_scalar(
                        ptv[:, :, g * R : (g + 1) * R],
                        ap_ps[g][:].rearrange("p (two r) -> p two r", two=2),
                        PSUM_DESC, None, OP.mult,
                    )
                return pt

            def emit_PB(pt, bx):
                ptb = gpool.tile([128, NG * 2 * R], BF16, tag="PB",
                                 name="PB0")
                nc.vector.tensor_tensor(ptb[:], pt[:], bx[:], OP.add)
                return ptb

            def emit_comb(bx, t0):
                # pairs 1-3: bexp is ready a full pair ahead, so one stt per g
                # evicts psum*c + bexp directly -- and the M-chain drops the
                # gpsimd bexp-add from its critical path.
                ptb = gpool.tile([128, NG * 2 * R], BF16, tag="PB",
                                 name=f"PB{t0}")
                ptv = ptb[:].rearrange("p (two gr) -> p two gr", two=2)
                bxv = bx[:].rearrange("p (two gr) -> p two gr", two=2)
                for g in range(NG):
                    nc.vector.scalar_tensor_tensor(
                        ptv[:, :, g * R : (g + 1) * R],
                        ap_ps[g][:].rearrange("p (two r) -> p two r", two=2),
                        PSUM_DESC,
                        bxv[:, :, g * R : (g + 1) * R],
                        OP.mult, OP.add,
                    )
                return ptb

            def emit_m_chain(ptb, sub, t):
                th0 = float(2 ** (t + 1))
                # M_t = (P' + bexp)_t + Mm  where Mm = q8*M_{t-1} (gpsimd)
                nc.vector.tensor_tensor(
                    M[:], ptb[:, sub * NG * R : (sub + 1) * NG * R], Mm[:],
                    OP.add)
                # spike (W1 input) first: it gates the PE
                spq = qpool.tile([128, NG * R], FP8, tag="spq", name=f"spq_{t}")
                nc.vector.tensor_scalar(spq[:], M[:], th0, None, OP.is_gt)
                if t < T - 1:
                    q8 = qpool.tile([128, NG * R], FP8, tag="q8",
                                    name=f"q8_{t}")
                    nc.vector.tensor_scalar(q8[:], M[:], th0, None, OP.is_le)
                    nc.gpsimd.tensor_tensor(Mm[:], q8[:], M[:], OP.mult)
                return spq

            def emit_w1(t, spq):
                # Per h-half: Act preloads psum with 0.5*Y_{t-1} (exact Copy),
                # W1 DR matmuls (negated wall) accumulate.
                # psum_t = -W_SC/2^t * ml_t.
                hqs = []
                for h in range(2):
                    hq = hpool.tile([128, 2 * R], F32, tag=f"hq{h}",
                                    name=f"hq{h}_{t}")
                    hqs.append(hq)
                    nc.scalar.activation(hq[:], Yh[h][:], ACT_COPY, scale=0.5)
                    for g in (2 * h, 2 * h + 1):
                        for kk in range(NH2):
                            rhs = spq[:, kk * 2 * R : (kk + 1) * 2 * R
                                      ].rearrange("p (two r) -> p two r", two=2)
                            nc.tensor.matmul(
                                hq[:, (g - 2 * h) * R : (g - 2 * h + 1) * R],
                                lhsT=w1T(kk, g),
                                rhs=rhs,
                                start=False,
                                stop=(kk == NH2 - 1),
                                perf_mode=DRMODE,
                                skip_group_check=True,
                            )
                return hqs

            def emit_ml_evict(t, hqs):
                # sp2 = (ml > 2^t) <=> (psum < -W_SC); Y = (sp2==0)*psum is
                # the masked carry (stt allows only one PSUM operand).
                sp2 = qpool.tile([128, NG * R], FP8, tag="sp2", name=f"sp2_{t}")
                for h in range(2):
                    s2h = sp2[:, h * 2 * R : (h + 1) * 2 * R]
                    nc.vector.tensor_scalar(s2h, hqs[h][:], -W_SC, None,
                                            OP.is_lt)
                    if t < T - 1:
                        nc.vector.scalar_tensor_tensor(
                            Yh[h][:], s2h, 0.0, hqs[h][:], OP.is_equal, OP.mult,
                        )
                return sp2

            def emit_w2(t, sp2):
                for kk in range(2):
                    nc.tensor.matmul(
                        o_psum[:],
                        lhsT=w2T(kk),
                        rhs=sp2[:, kk * 2 * R : (kk + 1) * 2 * R].rearrange(
                            "p (two r) -> p two r", two=2
                        ),
                        start=(t == 0 and kk == 0),
                        stop=(t == T - 1 and kk == 1),
                        perf_mode=DRMODE,
                    )

            # ---- prologue: pair-0 apical (DMA-paced), then basal ----
            for c in range(NCHUNK):
                emit_ap_chunk(te0_tiles, c)
            emit_basal()
            bx0 = gpool.tile([128, NG * 2 * R], BF16, tag="bx", name="bx0")
            emit_bexp(bx0, 0, 0)
            emit_bexp(bx0, 1, 1)

            # ---- software-pipelined main loop ----
            # Defer each pair's t1 layer-1/2 work into the next pair's
            # emission so PE reaches the next pair's apical matmuls promptly.
            carry = None           # (t1, spqb) of the previous pair
            bx = bx0
            for pair in range(NPAIR):
                t0, t1 = 2 * pair, 2 * pair + 1
                if pair == 0:
                    pt = emit_P(t0)
                    ptb = emit_PB(pt, bx)
                else:
                    ptb = emit_comb(bx, t0)
                spqa = emit_m_chain(ptb, 0, t0)
                if pair + 1 < NPAIR:
                    te_tiles = emit_te_dma(pair + 1)
                    emit_ap_chunk(te_tiles, 0)
                    if carry is not None:
                        tp, spqp = carry
                        hqp = emit_w1(tp, spqp)
                        sp2p = emit_ml_evict(tp, hqp)
                        emit_w2(tp, sp2p)
                    emit_ap_chunk(te_tiles, 1)
                    bxn = gpool.tile([128, NG * 2 * R], BF16, tag="bx",
                                     name=f"bx{pair + 1}")
                    emit_bexp(bxn, 0, t0 + 2)
                    emit_bexp(bxn, 1, t1 + 2)
                    spqb = emit_m_chain(ptb, 1, t1)
                    emit_ap_chunk(te_tiles, 2)
                    hqa = emit_w1(t0, spqa)
                    sp2a = emit_ml_evict(t0, hqa)
                    emit_ap_chunk(te_tiles, 3)
                    emit_ap_chunk(te_tiles, 4)
                    emit_w2(t0, sp2a)
                    bx = bxn
                    carry = (t1, spqb)
                else:
                    if carry is not None:
                        tp, spqp = carry
                        hqp = emit_w1(tp, spqp)
                        sp2p = emit_ml_evict(tp, hqp)
                        emit_w2(tp, sp2p)
                    hqa = emit_w1(t0, spqa)
                    spqb = emit_m_chain(ptb, 1, t1)
                    sp2a = emit_ml_evict(t0, hqa)
                    emit_w2(t0, sp2a)
                    hqb = emit_w1(t1, spqb)
                    sp2b = emit_ml_evict(t1, hqb)
                    emit_w2(t1, sp2b)

            # ---- final eviction: out = o_psum / (T*W_SC) + b2 ----
            out_sb = state.tile([L, R], F32, tag="out_sb", name="out_sb")
            nc.scalar.activation(
                out_sb[:], o_psum[:], ACT_ID,
                bias=evb_ap, scale=1.0 / (T * W_SC),
            )
            nc.sync.dma_start(out[:], out_sb[:])

    return nc


def _swizzle_dr(a, cols):
    """[KD, cols] fp -> fp8 [128, nk*cols]: [p, kt*cols + n] = a[kt*128+p, n]"""
    f8 = ml_dtypes.float8_e4m3
    nk = a.shape[0] // 128
    return np.ascontiguousarray(
        a.reshape(nk, 128, cols).transpose(1, 0, 2).reshape(128, nk * cols).astype(f8)
    )


def prep_in_maps(inputs):
    """Host-side shard + transpose + pad + scale + cumsum + cast."""
    se = np.asarray(inputs["state_embedding"], np.float32)
    te = np.asarray(inputs["tau_embedding"], np.float32)
    Wb = np.asarray(inputs["Wb"], np.float32)
    Wa = np.asarray(inputs["Wa"], np.float32)
    W1 = np.asarray(inputs["W1"], np.float32)
    b1 = np.asarray(inputs["b1"], np.float32)
    W2 = np.asarray(inputs["W2"], np.float32)
    b2 = np.asarray(inputs["b2"], np.float32)
    f8 = ml_dtypes.float8_e4m3

    def padk(a):  # pad feature axis 0 from 3136 to KD
        o = np.zeros((KD,) + a.shape[1:], a.dtype)
        o[: a.shape[0]] = a
        return o

    tsc = (2.0 ** (np.arange(T) - 1)) * EMB_SC   # fold 2^{t-1} into embeddings

    wallA = _swizzle_dr(padk(Wa.T) * W_SC, F)
    wallM_wb = _swizzle_dr(padk(Wb.T) * W_SC, F)

    # W1 wall: 4 k-tiles of -W1^T*512 (negated so the single-scale psum
    # eviction yields ml = mlmask + 2^t*(W1@sp); b1 == 0 in this problem)
    assert not np.any(b1), "nonzero b1 needs an extra constant k-pair"
    wallB = _swizzle_dr(np.ascontiguousarray(W1.T) * -W_SC, H)  # [128, 4*H]

    # W2 wall: [128, kt*LP + l] = W2[l, kt*128+p] * W_SC, 4 k-tiles (2 DR pairs)
    w2w = np.zeros((128, W2_COLS), f8)
    for kt in range(4):
        w2w[:, kt * LP : kt * LP + L] = (W2.T[kt * 128 : (kt + 1) * 128] * W_SC
                                         ).astype(f8)

    cons = np.zeros((128, 1), np.float32)
    cons[:L, 0] = b2

    in_maps = []
    for i in range(N_CORES):
        # teT: [pair, p, kt*2R + sub*R + r] = cumsum_t(te * 2^{t-1} * EMB_SC)
        tei = te[:, i * R : (i + 1) * R, :] * tsc[:, None, None]  # [T, R, DT]
        tei = np.cumsum(tei, axis=0)
        tei = tei.reshape(NPAIR, 2 * R, DT)
        tei_p = np.zeros((NPAIR, 2 * R, KD), np.float32)
        tei_p[:, :, :DT] = tei
        teT = np.ascontiguousarray(
            tei_p.reshape(NPAIR, 2 * R, NK2 * 2, 128)
            .transpose(0, 3, 2, 1)                # [pair, p, ktile, n]
            .reshape(NPAIR, 128, NK2 * 2 * 2 * R)
            .astype(f8)
        )
        # seT region: cumsum over t as well (binc becomes cumulative basal)
        sei = se[:, i * NB : (i + 1) * NB, :] * tsc[:, None, None]  # [T, NB, DS]
        sei = np.cumsum(sei, axis=0)
        seT = padk(np.ascontiguousarray(sei.reshape(T * NB, DS).T))  # [KD, T*NB]
        wallM_i = np.empty((128, WM_COLS), f8)
        wallM_i[:, O_WB : O_WB + NK2 * 2 * F] = wallM_wb
        wallM_i[:, O_SE :] = _swizzle_dr(seT, T * NB)
        in_maps.append(dict(teT=teT, wallA=wallA, wallM=wallM_i, wallB=wallB,
                            w2w=w2w, cons=cons))
    return in_maps


def assemble_out(core_outs):
    """[N_CORES][L, R] -> [B, L, S]"""
    full = np.stack([np.asarray(o, np.float32) for o in core_outs], axis=0)
    full = full.reshape(N_CORES, L, NB, S).transpose(0, 2, 1, 3)
    return np.ascontiguousarray(full.reshape(B, L, S))


_NC_CACHE = {}


def get_nc():
    if "nc" not in _NC_CACHE:
        last = None
        for _ in range(6):
            try:
                _NC_CACHE["nc"] = build_nc()
                break
            except Exception as e:  # rare scheduler-order race-detector trip
                last = e
        else:
            raise last
    return _NC_CACHE["nc"]


def run_sharded(in_maps, trace=False, **kw):
    nc = get_nc()
    if not getattr(nc, "_waits_split", False):
        _split_excess_waits(nc)
        nc._waits_split = True
    res = run_bass_kernel_spmd(
        nc, in_maps, core_ids=list(range(N_CORES)), trace=trace, **kw
    )
    return res


def kernel(**inputs):
    in_maps = prep_in_maps(inputs)
    res = run_sharded(in_maps)
    return assemble_out([res.results[i]["out"] for i in range(N_CORES)])


# revision 47
# speedup vs baseline: 1.0189x; 1.0189x over previous
"""Trainium2 Bass kernel for nn_MCQuantiles (ThreeCompNode SNN scan).

Strategy (8 NeuronCores, data-parallel over batch):
- Each core takes 8 batches x 32 samples = 256 rows of the B*S axis.
- Everything runs in "transposed space": feature dims on SBUF partitions,
  batch-rows on the free dim. All transposes/swizzles/casts are host-side;
  every DMA is a flat contiguous [128, X] block.
- All matmuls run in fp8(e4m3) DoubleRow mode: 2 k-tiles per PE pass = 2x
  bf16 throughput.
- KEY RESTRUCTURE vs v1: G_t = 2^t(ma_t+mb_t) = sum_{s<=t} 2^{s-1}(ap_s+ba_s)
  is LINEAR in the embeddings, so the time-cumsum is folded into te/se on the
  HOST (tec_t = cumsum of 2^{t-1}*te*EMB_SC). One matmul then yields G_t
  directly -- no G recurrence on device:
      P_t  = ap_psum_t * c + bexpcum_t          (bexpcum = cumulative basal)
      M_t  = P_t + Mmask_{t-1},  q = (M <= 2^{t+1}),  Mmask = q*M (gpsimd)
- Layer-1 feeds W1 with the spike sp (not NOT-spike): with b1 == 0 there is
  then NO derived constant, so the hq psum PRELOAD is a single exact Act
  Copy (-W_SC/2^t)*mlmask_{t-1} (the table-based Identity+bias path clips
  large inputs on hardware -- avoid it). W1 wall is negated host-side so the
  one-scale eviction ml_t = psum * (-2^t/W_SC) = mlmask + 2^t*(W1@sp) comes
  out with the right sign. The whole ML update runs on the (otherwise idle)
  Act engine; DVE does the compares + ml mask. (A nonzero b1 would use one
  extra constant k-pair, baseline-style.)
- Layer-2 feeds sp2 directly into W2 (fp8): zero spikes accumulate an
  exactly-zero psum, preserving the bit-exact b2 output.
- ap psum is 4 per-g single-bank tiles; the per-g stt eviction releases each
  bank individually so the next pair's g-matmuls restart sooner.
- Startup DMA triggers are spread across engine queues (sync/scalar/vector/
  gpsimd) instead of serializing ~10 x 640ns on the Sync queue.
"""
import numpy as np
import ml_dtypes

import bass_rust
import concourse.bass as bass
import concourse.mybir as mybir
from concourse.bass_utils import run_bass_kernel_spmd
from concourse.tile import TileContext
from concourse.tile_rust import add_dep_helper

# ----- problem constants (hardcoded per contract) -----
T, B, S = 8, 64, 32
DS = DT = 3136
F = H = 512
L = 18
N_CORES = 8
NB = B // N_CORES              # 8 batches per core
R = NB * S                     # 256 rows per core
KD = 3328                      # 3136 padded to 26 k-tiles of 128
NK2 = KD // 256                # 13 DoubleRow k-pairs
NPAIR = T // 2                 # 4 step pairs
NG = F // 128                  # 4 f-tiles (= h-tiles)
NH2 = 2                        # W1 contraction: 2 DR pairs (c1 via psum preload)

WA_COLS = NK2 * 2 * F          # fp8 apical weight wall
O_WB = 0                       # wallM: basal weights
O_SE = NK2 * 2 * F             # then state embeddings
WM_COLS = O_SE + NK2 * 2 * T * NB
WB_COLS = NH2 * 2 * H          # fp8 W1 wall (2 DR pairs only)
LP = 32                        # W2 k-tile column pitch (L=18 padded for align)
W2_COLS = 2 * 2 * LP           # fp8 W2 wall, 2 DR pairs

# scales folded host-side (see prep_in_maps)
EMB_SC = 0.25                  # global embedding scale (te/se * 2^{t-1} * EMB_SC)
W_SC = 512.0                   # weight scale for Wa/Wb/W1/W2
PSUM_DESC = 1.0 / (EMB_SC * W_SC)   # apical/basal psum -> G increment

F32 = mybir.dt.float32
BF16 = mybir.dt.bfloat16
FP8 = mybir.dt.float8e4
OP = mybir.AluOpType
DRMODE = mybir.MatmulPerfMode.DoubleRow
ACT_COPY = mybir.ActivationFunctionType.Copy
ACT_ID = mybir.ActivationFunctionType.Identity

# te DMA chunking in DR k-pair units (13 total); first small for startup
CHUNKS = [1, 2, 2, 4, 4]
CH_OFF = [0, 1, 3, 5, 9]
NCHUNK = len(CHUNKS)


def _patch_tile_drain():
    """This walrus build allows a single sync-wait per TPB_CTRL Drain; Tile's
    kernel-tail drain attaches one wait per active logical proc. Split them
    across a chain of drains."""
    def _patched(self, tick_clock, wait_clock):
        nc = self.nc
        drain_inst = nc.sync.drain()
        wait_clock.add_sem_waits(
            drain_inst.ins, bass_rust.ScopedClock({None: tick_clock.global_clock})
        )
        si = drain_inst.ins.sync_info
        if si is not None and len(si.on_wait) > 1:
            waits = list(si.on_wait)
            drain_inst.ins.sync_info = mybir.SyncInfo(
                on_wait=waits[:1], on_update=list(si.on_update)
            )
            for w in waits[1:]:
                extra = nc.sync.drain()
                extra.ins.sync_info = mybir.SyncInfo(on_wait=[w], on_update=[])
        nc.all_engine_barrier()
        popped = nc._tile_sem_poison_stack.pop()
        assert popped is self._sem_poison
        nc.clear_and_free_semaphores(list(self.sems.allocated().values()))
        nc.all_engine_barrier()

    TileContext._drain_and_barrier = _patched


def _split_excess_waits(nc, limit=1):
    """Walrus here rejects instructions carrying more than ~1 sync-wait. Move
    excess waits onto same-engine NoOps inserted just before the instruction."""
    for fn in nc.m.functions:
        for bb in fn.blocks:
            new = []
            changed = False
            for inst in bb.instructions:
                si = getattr(inst, "sync_info", None)
                ow = list(si.on_wait) if si is not None and si.on_wait else []
                if len(ow) > limit:
                    extra = ow[limit:]
                    for j in range(0, len(extra), limit):
                        nop = mybir.InstNoOp(
                            name=f"{inst.name}-ws{j}", ins=[], outs=[]
                        )
                        nop.engine = inst.engine
                        nop.sync_info = mybir.SyncInfo(
                            on_wait=extra[j : j + limit], on_update=[]
                        )
                        new.append(nop)
                    inst.sync_info = mybir.SyncInfo(
                        on_wait=ow[:limit], on_update=list(si.on_update)
                    )
                    changed = True
                new.append(inst)
            if changed:
                try:
                    bb.instructions[:] = new
                except TypeError:
                    bb.instructions = new


def build_nc():
    _patch_tile_drain()
    nc = bass.Bass()

    teT = nc.declare_dram_parameter("teT", [NPAIR, 128, NK2 * 2 * 2 * R], FP8,
                                    isOutput=False)
    wallA = nc.declare_dram_parameter("wallA", [128, WA_COLS], FP8, isOutput=False)
    wallM = nc.declare_dram_parameter("wallM", [128, WM_COLS], FP8, isOutput=False)
    wallB = nc.declare_dram_parameter("wallB", [128, WB_COLS], FP8, isOutput=False)
    w2w = nc.declare_dram_parameter("w2w", [128, W2_COLS], FP8, isOutput=False)
    cons = nc.declare_dram_parameter("cons", [128, 1], F32, isOutput=False)
    out = nc.declare_dram_parameter("out", [L, R], F32, isOutput=True)

    with TileContext(nc) as tc:
        with (
            tc.tile_pool(name="wpool", bufs=1) as wpool,
            tc.tile_pool(name="tepool", bufs=2) as tepool,
            tc.tile_pool(name="state", bufs=1) as state,
            tc.tile_pool(name="qpool", bufs=3) as qpool,
            tc.tile_pool(name="gpool", bufs=2) as gpool,
            tc.tile_pool(name="appool", bufs=1, space="PSUM") as appool,
            tc.tile_pool(name="hpool", bufs=1, space="PSUM") as hpool,
            tc.tile_pool(name="bpool", bufs=1, space="PSUM") as bpool,
            tc.tile_pool(name="opool", bufs=1, space="PSUM") as opool,
        ):
            # ---- startup DMAs: spread across engine queues so descriptor
            # generation parallelizes; order within each queue by need-time.
            wallM_sb = wpool.tile([128, WM_COLS], FP8, tag="wallM", name="wallM_sb")
            wallB_sb = wpool.tile([128, WB_COLS], FP8, tag="wallB", name="wallB_sb")
            w2_sb = wpool.tile([128, W2_COLS], FP8, tag="w2w", name="w2_sb")
            cons_sb = wpool.tile([128, 1], F32, tag="cons", name="cons_sb")

            def te_dma(tck, pair, c):
                nc.sync.dma_start(
                    tck[:],
                    teT[pair][:, CH_OFF[c] * 4 * R
                              : (CH_OFF[c] + CHUNKS[c]) * 4 * R],
                )

            te0_tiles = []
            wallA_c = []
            for c in range(NCHUNK):
                tck = tepool.tile(
                    [128, CHUNKS[c] * 2 * 2 * R], FP8, tag=f"te{c}",
                    name=f"te_ck{c}",
                )
                te0_tiles.append(tck)
                te_dma(tck, 0, c)
                wa_ck = wpool.tile(
                    [128, CHUNKS[c] * 2 * F], FP8, tag=f"wallA{c}", name=f"wa_ck{c}"
                )
                wallA_c.append(wa_ck)
                nc.scalar.dma_start(
                    wa_ck[:],
                    wallA[:, CH_OFF[c] * 2 * F : (CH_OFF[c] + CHUNKS[c]) * 2 * F],
                )
            nc.gpsimd.dma_start(wallM_sb[:], wallM[:])
            nc.gpsimd.dma_start(wallB_sb[:], wallB[:])
            nc.gpsimd.dma_start(w2_sb[:], w2w[:])
            nc.gpsimd.dma_start(cons_sb[:], cons[:])

            def waT(kk, g):
                # lhsT [128, 2, 128] for DR pair kk, out tile g
                for c in range(NCHUNK - 1, -1, -1):
                    if kk >= CH_OFF[c]:
                        k = kk - CH_OFF[c]
                        v = wallA_c[c][:, k * 2 * F : (k + 1) * 2 * F].rearrange(
                            "p (two f) -> p two f", two=2
                        )
                        return v[:, :, g * 128 : (g + 1) * 128]

            def wbT(kk, g):
                v = wallM_sb[:, O_WB + kk * 2 * F : O_WB + (kk + 1) * 2 * F]
                v = v.rearrange("p (two f) -> p two f", two=2)
                return v[:, :, g * 128 : (g + 1) * 128]

            def seT(kk):
                v = wallM_sb[:, O_SE + kk * 2 * T * NB : O_SE + (kk + 1) * 2 * T * NB]
                return v.rearrange("p (two n) -> p two n", two=2)

            def w1T(kk, g):
                v = wallB_sb[:, kk * 2 * H : (kk + 1) * 2 * H].rearrange(
                    "p (two h) -> p two h", two=2
                )
                return v[:, :, g * 128 : (g + 1) * 128]

            def w2T(kk):
                v = w2_sb[:, kk * 2 * LP : (kk + 1) * 2 * LP]
                v = v.rearrange("p (two l) -> p two l", two=2)
                return v[:, :, 0:L]

            evb_ap = cons_sb[0:L, 0:1]

            # ---- state tiles ----
            M = state.tile([128, NG * R], BF16, tag="M", name="M")
            Mm = state.tile([128, NG * R], BF16, tag="Mm", name="Mm")
            # Y = masked layer-1 psum carry: Y_t = psum_t * (psum_t >= -W_SC);
            # psum_{t+1} = 0.5*Y_t + W_SC*(-W1)@sp  (thresholds double per t).
            # Split in h-halves so each half's preload->W1->compare chain
            # pipelines independently (tile-granular deps).
            Yh = [state.tile([128, 2 * R], BF16, tag=f"Y{h}", name=f"Y{h}")
                  for h in range(2)]
            binc = state.tile([128, NG * T * NB], BF16, tag="binc", name="binc")

            # ap psum: one single-bank tile per g so evictions release banks
            # individually
            ap_ps = [appool.tile([128, 2 * R], F32, tag=f"ap{g}", name=f"ap{g}")
                     for g in range(NG)]
            o_psum = opool.tile([L, R], F32, tag="o", name="o_psum")

            nc.vector.memset(Yh[0][:], 0.0)
            nc.vector.memset(Yh[1][:], 0.0)

            # ---- te DMA + apical matmul emission ----
            def emit_te_dma(pair):
                tiles = []
                for c in range(NCHUNK):
                    tck = tepool.tile(
                        [128, CHUNKS[c] * 2 * 2 * R], FP8, tag=f"te{c}",
                        name=f"te_ck{c}",
                    )
                    tiles.append(tck)
                    te_dma(tck, pair, c)
                return tiles

            def emit_ap_chunk(te_tiles, c):
                for g in range(NG):
                    for k in range(CHUNKS[c]):
                        kk = CH_OFF[c] + k
                        rhs = te_tiles[c][:, k * 4 * R : (k + 1) * 4 * R].rearrange(
                            "p (two n) -> p two n", two=2
                        )
                        nc.tensor.matmul(
                            ap_ps[g][:],
                            lhsT=waT(kk, g),
                            rhs=rhs,
                            start=(kk == 0),
                            stop=(kk == NK2 - 1),
                            perf_mode=DRMODE,
                        )

            def emit_basal():
                bs_psum = bpool.tile([128, NG * T * NB], F32, tag="bs",
                                     name="bs_psum")
                for g in range(NG):
                    for kk in range(NK2):
                        nc.tensor.matmul(
                            bs_psum[:, g * T * NB : (g + 1) * T * NB],
                            lhsT=wbT(kk, g),
                            rhs=seT(kk),
                            start=(kk == 0),
                            stop=(kk == NK2 - 1),
                            perf_mode=DRMODE,
                        )
                # binc (g-major) = cumulative basal contribution to G
                nc.scalar.activation(binc[:], bs_psum[:], ACT_COPY,
                                     scale=PSUM_DESC)

            def binc_bc(t):
                v = binc[:].rearrange("p (g x) -> p g x", g=NG)
                v = v[:, :, t * NB : (t + 1) * NB]
                return v.unsqueeze(3).broadcast_to([128, NG, NB, S])

            def emit_bexp(bx, sub, t):
                # cumulative-basal expansion for timestep t into pair tile
                # half; bx layout is (sub, g, r)
                nc.scalar.activation(
                    bx[:, sub * NG * R : (sub + 1) * NG * R].rearrange(
                        "p (g b s) -> p g b s", g=NG, s=S),
                    binc_bc(t), ACT_COPY,
                )

            # ---- per-pair state math ----
            def emit_P(t0):
                # P' = ap_psum * c, one ts per g: releases psum bank g with NO
                # dependency on basal/bexp -- the next pair's matmuls restart
                # immediately. The basal term rides in Mm (see emit_m_chain).
                pt = gpool.tile([128, NG * 2 * R], BF16, tag="P", name=f"P{t0}")
                ptv = pt[:].rearrange("p (two gr) -> p two gr", two=2)
                for g in range(NG):
                    nc.vector.tensor_scalar(
                        ptv[:, :, g * R : (g + 1) * R],
                        ap_ps[g][:].rearrange("p (two r) -> p two r", two=2),
                        PSUM_DESC, None, OP.mult,
                    )
                return pt

            def bxs(bx, sub):
                return bx[:, sub * NG * R : (sub + 1) * NG * R]

            def emit_m_chain(pt, sub, t, bx_next, sub_next):
                th0 = float(2 ** (t + 1))
                # M_t = P'_t + Mm  where Mm = q8*M_{t-1} + bexp_t (gpsimd)
                nc.vector.tensor_tensor(
                    M[:], pt[:, sub * NG * R : (sub + 1) * NG * R], Mm[:],
                    OP.add)
                # spike (W1 input) first: it gates the PE
                spq = qpool.tile([128, NG * R], FP8, tag="spq", name=f"spq_{t}")
                nc.vector.tensor_scalar(spq[:], M[:], th0, None, OP.is_gt)
                if t < T - 1:
                    q8 = qpool.tile([128, NG * R], FP8, tag="q8",
                                    name=f"q8_{t}")
                    nc.vector.tensor_scalar(q8[:], M[:], th0, None, OP.is_le)
                    # Mm = q8*M + bexp_{t+1}, off the DVE queue
                    nc.gpsimd.tensor_tensor(Mm[:], q8[:], M[:], OP.mult)
                    nc.gpsimd.tensor_tensor(Mm[:], Mm[:], bxs(bx_next, sub_next),
                                            OP.add)
                return spq

            def emit_w1(t, spq):
                # Per h-half: Act preloads psum with 0.5*Y_{t-1} (exact Copy),
                # W1 DR matmuls (negated wall) accumulate.
                # psum_t = -W_SC/2^t * ml_t.
                hqs = []
                for h in range(2):
                    hq = hpool.tile([128, 2 * R], F32, tag=f"hq{h}",
                                    name=f"hq{h}_{t}")
                    hqs.append(hq)
                    nc.scalar.activation(hq[:], Yh[h][:], ACT_COPY, scale=0.5)
                    for g in (2 * h, 2 * h + 1):
                        for kk in range(NH2):
                            rhs = spq[:, kk * 2 * R : (kk + 1) * 2 * R
                                      ].rearrange("p (two r) -> p two r", two=2)
                            nc.tensor.matmul(
                                hq[:, (g - 2 * h) * R : (g - 2 * h + 1) * R],
                                lhsT=w1T(kk, g),
                                rhs=rhs,
                                start=False,
                                stop=(kk == NH2 - 1),
                                perf_mode=DRMODE,
                                skip_group_check=True,
                            )
                return hqs

            def emit_ml_evict(t, hqs):
                # sp2 = (ml > 2^t) <=> (psum < -W_SC); Y = (sp2==0)*psum is
                # the masked carry (stt allows only one PSUM operand).
                sp2 = qpool.tile([128, NG * R], FP8, tag="sp2", name=f"sp2_{t}")
                for h in range(2):
                    s2h = sp2[:, h * 2 * R : (h + 1) * 2 * R]
                    nc.vector.tensor_scalar(s2h, hqs[h][:], -W_SC, None,
                                            OP.is_lt)
                    if t < T - 1:
                        nc.vector.scalar_tensor_tensor(
                            Yh[h][:], s2h, 0.0, hqs[h][:], OP.is_equal, OP.mult,
                        )
                return sp2

            def emit_w2(t, sp2):
                for kk in range(2):
                    nc.tensor.matmul(
                        o_psum[:],
                        lhsT=w2T(kk),
                        rhs=sp2[:, kk * 2 * R : (kk + 1) * 2 * R].rearrange(
                            "p (two r) -> p two r", two=2
                        ),
                        start=(t == 0 and kk == 0),
                        stop=(t == T - 1 and kk == 1),
                        perf_mode=DRMODE,
                    )

            # ---- prologue: pair-0 apical (DMA-paced), then basal ----
            for c in range(NCHUNK):
                emit_ap_chunk(te0_tiles, c)
            emit_basal()
            bx0 = gpool.tile([128, NG * 2 * R], BF16, tag="bx", name="bx0")
            emit_bexp(bx0, 0, 0)
            emit_bexp(bx0, 1, 1)
            # M_0 = P'_0 + Mm with Mm initialized to bexp_0
            nc.scalar.activation(Mm[:], bx0[:, 0 : NG * R], ACT_COPY)

            # ---- software-pipelined main loop ----
            # Defer each pair's t1 layer-1/2 work into the next pair's
            # emission so PE reaches the next pair's apical matmuls promptly.
            carry = None           # (t1, spqb) of the previous pair
            bx = bx0
            for pair in range(NPAIR):
                t0, t1 = 2 * pair, 2 * pair + 1
                pt = emit_P(t0)
                spqa = emit_m_chain(pt, 0, t0, bx, 1)
                if pair + 1 < NPAIR:
                    te_tiles = emit_te_dma(pair + 1)
                    emit_ap_chunk(te_tiles, 0)
                    if carry is not None:
                        tp, spqp = carry
                        hqp = emit_w1(tp, spqp)
                        sp2p = emit_ml_evict(tp, hqp)
                        emit_w2(tp, sp2p)
                    emit_ap_chunk(te_tiles, 1)
                    bxn = gpool.tile([128, NG * 2 * R], BF16, tag="bx",
                                     name=f"bx{pair + 1}")
                    emit_bexp(bxn, 0, t0 + 2)
                    emit_bexp(bxn, 1, t1 + 2)
                    spqb = emit_m_chain(pt, 1, t1, bxn, 0)
                    emit_ap_chunk(te_tiles, 2)
                    hqa = emit_w1(t0, spqa)
                    sp2a = emit_ml_evict(t0, hqa)
                    emit_ap_chunk(te_tiles, 3)
                    emit_ap_chunk(te_tiles, 4)
                    emit_w2(t0, sp2a)
                    bx = bxn
                    carry = (t1, spqb)
                else:
                    if carry is not None:
                        tp, spqp = carry
                        hqp = emit_w1(tp, spqp)
                        sp2p = emit_ml_evict(tp, hqp)
                        emit_w2(tp, sp2p)
                    hqa = emit_w1(t0, spqa)
                    spqb = emit_m_chain(pt, 1, t1, None, None)
                    sp2a = emit_ml_evict(t0, hqa)
                    emit_w2(t0, sp2a)
                    hqb = emit_w1(t1, spqb)
                    sp2b = emit_ml_evict(t1, hqb)
                    emit_w2(t1, sp2b)

            # ---- final eviction: out = o_psum / (T*W_SC) + b2 ----
            out_sb = state.tile([L, R], F32, tag="out_sb", name="out_sb")
            nc.scalar.activation(
                out_sb[:], o_psum[:], ACT_ID,
                bias=evb_ap, scale=1.0 / (T * W_SC),
            )
            nc.sync.dma_start(out[:], out_sb[:])

    return nc


def _swizzle_dr(a, cols):
    """[KD, cols] fp -> fp8 [128, nk*cols]: [p, kt*cols + n] = a[kt*128+p, n]"""
    f8 = ml_dtypes.float8_e4m3
    nk = a.shape[0] // 128
    return np.ascontiguousarray(
        a.reshape(nk, 128, cols).transpose(1, 0, 2).reshape(128, nk * cols).astype(f8)
    )


def prep_in_maps(inputs):
    """Host-side shard + transpose + pad + scale + cumsum + cast."""
    se = np.asarray(inputs["state_embedding"], np.float32)
    te = np.asarray(inputs["tau_embedding"], np.float32)
    Wb = np.asarray(inputs["Wb"], np.float32)
    Wa = np.asarray(inputs["Wa"], np.float32)
    W1 = np.asarray(inputs["W1"], np.float32)
    b1 = np.asarray(inputs["b1"], np.float32)
    W2 = np.asarray(inputs["W2"], np.float32)
    b2 = np.asarray(inputs["b2"], np.float32)
    f8 = ml_dtypes.float8_e4m3

    def padk(a):  # pad feature axis 0 from 3136 to KD
        o = np.zeros((KD,) + a.shape[1:], a.dtype)
        o[: a.shape[0]] = a
        return o

    tsc = (2.0 ** (np.arange(T) - 1)) * EMB_SC   # fold 2^{t-1} into embeddings

    wallA = _swizzle_dr(padk(Wa.T) * W_SC, F)
    wallM_wb = _swizzle_dr(padk(Wb.T) * W_SC, F)

    # W1 wall: 4 k-tiles of -W1^T*512 (negated so the single-scale psum
    # eviction yields ml = mlmask + 2^t*(W1@sp); b1 == 0 in this problem)
    assert not np.any(b1), "nonzero b1 needs an extra constant k-pair"
    wallB = _swizzle_dr(np.ascontiguousarray(W1.T) * -W_SC, H)  # [128, 4*H]

    # W2 wall: [128, kt*LP + l] = W2[l, kt*128+p] * W_SC, 4 k-tiles (2 DR pairs)
    w2w = np.zeros((128, W2_COLS), f8)
    for kt in range(4):
        w2w[:, kt * LP : kt * LP + L] = (W2.T[kt * 128 : (kt + 1) * 128] * W_SC
                                         ).astype(f8)

    cons = np.zeros((128, 1), np.float32)
    cons[:L, 0] = b2

    in_maps = []
    for i in range(N_CORES):
        # teT: [pair, p, kt*2R + sub*R + r] = cumsum_t(te * 2^{t-1} * EMB_SC)
        tei = te[:, i * R : (i + 1) * R, :] * tsc[:, None, None]  # [T, R, DT]
        tei = np.cumsum(tei, axis=0)
        tei = tei.reshape(NPAIR, 2 * R, DT)
        tei_p = np.zeros((NPAIR, 2 * R, KD), np.float32)
        tei_p[:, :, :DT] = tei
        teT = np.ascontiguousarray(
            tei_p.reshape(NPAIR, 2 * R, NK2 * 2, 128)
            .transpose(0, 3, 2, 1)                # [pair, p, ktile, n]
            .reshape(NPAIR, 128, NK2 * 2 * 2 * R)
            .astype(f8)
        )
        # seT region: cumsum over t as well (binc becomes cumulative basal)
        sei = se[:, i * NB : (i + 1) * NB, :] * tsc[:, None, None]  # [T, NB, DS]
        sei = np.cumsum(sei, axis=0)
        seT = padk(np.ascontiguousarray(sei.reshape(T * NB, DS).T))  # [KD, T*NB]
        wallM_i = np.empty((128, WM_COLS), f8)
        wallM_i[:, O_WB : O_WB + NK2 * 2 * F] = wallM_wb
        wallM_i[:, O_SE :] = _swizzle_dr(seT, T * NB)
        in_maps.append(dict(teT=teT, wallA=wallA, wallM=wallM_i, wallB=wallB,
                            w2w=w2w, cons=cons))
    return in_maps


def assemble_out(core_outs):
    """[N_CORES][L, R] -> [B, L, S]"""
    full = np.stack([np.asarray(o, np.float32) for o in core_outs], axis=0)
    full = full.reshape(N_CORES, L, NB, S).transpose(0, 2, 1, 3)
    return np.ascontiguousarray(full.reshape(B, L, S))


_NC_CACHE = {}


def get_nc():
    if "nc" not in _NC_CACHE:
        last = None
        for _ in range(6):
            try:
                _NC_CACHE["nc"] = build_nc()
                break
            except Exception as e:  # rare scheduler-order race-detector trip
                last = e
        else:
            raise last
    return _NC_CACHE["nc"]


def run_sharded(in_maps, trace=False, **kw):
    nc = get_nc()
    if not getattr(nc, "_waits_split", False):
        _split_excess_waits(nc)
        nc._waits_split = True
    res = run_bass_kernel_spmd(
        nc, in_maps, core_ids=list(range(N_CORES)), trace=trace, **kw
    )
    return res


def kernel(**inputs):
    in_maps = prep_in_maps(inputs)
    res = run_sharded(in_maps)
    return assemble_out([res.results[i]["out"] for i in range(N_CORES)])


# revision 51
# speedup vs baseline: 1.0447x; 1.0253x over previous
"""Trainium2 Bass kernel for nn_MCQuantiles (ThreeCompNode SNN scan).

Strategy (8 NeuronCores, data-parallel over batch):
- Each core takes 8 batches x 32 samples = 256 rows of the B*S axis.
- Everything runs in "transposed space": feature dims on SBUF partitions,
  batch-rows on the free dim. All transposes/swizzles/casts are host-side;
  every DMA is a flat contiguous [128, X] block.
- All matmuls run in fp8(e4m3) DoubleRow mode: 2 k-tiles per PE pass = 2x
  bf16 throughput.
- KEY RESTRUCTURE vs v1: G_t = 2^t(ma_t+mb_t) = sum_{s<=t} 2^{s-1}(ap_s+ba_s)
  is LINEAR in the embeddings, so the time-cumsum is folded into te/se on the
  HOST (tec_t = cumsum of 2^{t-1}*te*EMB_SC). One matmul then yields G_t
  directly -- no G recurrence on device:
      P_t  = ap_psum_t * c + bexpcum_t          (bexpcum = cumulative basal)
      M_t  = P_t + Mmask_{t-1},  q = (M <= 2^{t+1}),  Mmask = q*M (gpsimd)
- Layer-1 feeds W1 with the spike sp (not NOT-spike): with b1 == 0 there is
  then NO derived constant, so the hq psum PRELOAD is a single exact Act
  Copy (-W_SC/2^t)*mlmask_{t-1} (the table-based Identity+bias path clips
  large inputs on hardware -- avoid it). W1 wall is negated host-side so the
  one-scale eviction ml_t = psum * (-2^t/W_SC) = mlmask + 2^t*(W1@sp) comes
  out with the right sign. The whole ML update runs on the (otherwise idle)
  Act engine; DVE does the compares + ml mask. (A nonzero b1 would use one
  extra constant k-pair, baseline-style.)
- Layer-2 feeds sp2 directly into W2 (fp8): zero spikes accumulate an
  exactly-zero psum, preserving the bit-exact b2 output.
- ap psum is 4 per-g single-bank tiles; the per-g stt eviction releases each
  bank individually so the next pair's g-matmuls restart sooner.
- Startup DMA triggers are spread across engine queues (sync/scalar/vector/
  gpsimd) instead of serializing ~10 x 640ns on the Sync queue.
"""
import numpy as np
import ml_dtypes

import bass_rust
import concourse.bass as bass
import concourse.mybir as mybir
from concourse.bass_utils import run_bass_kernel_spmd
from concourse.tile import TileContext
from concourse.tile_rust import add_dep_helper

# ----- problem constants (hardcoded per contract) -----
T, B, S = 8, 64, 32
DS = DT = 3136
F = H = 512
L = 18
N_CORES = 8
NB = B // N_CORES              # 8 batches per core
R = NB * S                     # 256 rows per core
KD = 3328                      # 3136 padded to 26 k-tiles of 128
NK2 = KD // 256                # 13 DoubleRow k-pairs
NPAIR = T // 2                 # 4 step pairs
NG = F // 128                  # 4 f-tiles (= h-tiles)
NH2 = 2                        # W1 contraction: 2 DR pairs (c1 via psum preload)

WA_COLS = NK2 * 2 * F          # fp8 apical weight wall
O_WB = 0                       # wallM: basal weights
O_SE = NK2 * 2 * F             # then state embeddings
WM_COLS = O_SE + NK2 * 2 * T * NB
WB_COLS = NH2 * 2 * H          # fp8 W1 wall (2 DR pairs only)
LP = 32                        # W2 k-tile column pitch (L=18 padded for align)
W2_COLS = 2 * 2 * LP           # fp8 W2 wall, 2 DR pairs

# scales folded host-side (see prep_in_maps)
EMB_SC = 0.25                  # global embedding scale (te/se * 2^{t-1} * EMB_SC)
W_SC = 512.0                   # weight scale for Wa/Wb/W1/W2
PSUM_DESC = 1.0 / (EMB_SC * W_SC)   # apical/basal psum -> G increment

F32 = mybir.dt.float32
BF16 = mybir.dt.bfloat16
FP8 = mybir.dt.float8e4
OP = mybir.AluOpType
DRMODE = mybir.MatmulPerfMode.DoubleRow
ACT_COPY = mybir.ActivationFunctionType.Copy
ACT_ID = mybir.ActivationFunctionType.Identity

# te DMA chunking in DR k-pair units (13 total); first small for startup
CHUNKS = [1, 2, 2, 4, 4]
CH_OFF = [0, 1, 3, 5, 9]
NCHUNK = len(CHUNKS)


def _patch_tile_drain():
    """This walrus build allows a single sync-wait per TPB_CTRL Drain; Tile's
    kernel-tail drain attaches one wait per active logical proc. Split them
    across a chain of drains."""
    def _patched(self, tick_clock, wait_clock):
        nc = self.nc
        drain_inst = nc.sync.drain()
        wait_clock.add_sem_waits(
            drain_inst.ins, bass_rust.ScopedClock({None: tick_clock.global_clock})
        )
        si = drain_inst.ins.sync_info
        if si is not None and len(si.on_wait) > 1:
            waits = list(si.on_wait)
            drain_inst.ins.sync_info = mybir.SyncInfo(
                on_wait=waits[:1], on_update=list(si.on_update)
            )
            for w in waits[1:]:
                extra = nc.sync.drain()
                extra.ins.sync_info = mybir.SyncInfo(on_wait=[w], on_update=[])
        nc.all_engine_barrier()
        popped = nc._tile_sem_poison_stack.pop()
        assert popped is self._sem_poison
        nc.clear_and_free_semaphores(list(self.sems.allocated().values()))
        nc.all_engine_barrier()

    TileContext._drain_and_barrier = _patched


def _split_excess_waits(nc, limit=1):
    """Walrus here rejects instructions carrying more than ~1 sync-wait. Move
    excess waits onto same-engine NoOps inserted just before the instruction."""
    for fn in nc.m.functions:
        for bb in fn.blocks:
            new = []
            changed = False
            for inst in bb.instructions:
                si = getattr(inst, "sync_info", None)
                ow = list(si.on_wait) if si is not None and si.on_wait else []
                if len(ow) > limit:
                    extra = ow[limit:]
                    for j in range(0, len(extra), limit):
                        nop = mybir.InstNoOp(
                            name=f"{inst.name}-ws{j}", ins=[], outs=[]
                        )
                        nop.engine = inst.engine
                        nop.sync_info = mybir.SyncInfo(
                            on_wait=extra[j : j + limit], on_update=[]
                        )
                        new.append(nop)
                    inst.sync_info = mybir.SyncInfo(
                        on_wait=ow[:limit], on_update=list(si.on_update)
                    )
                    changed = True
                new.append(inst)
            if changed:
                try:
                    bb.instructions[:] = new
                except TypeError:
                    bb.instructions = new


def build_nc():
    _patch_tile_drain()
    nc = bass.Bass()

    teT = nc.declare_dram_parameter("teT", [NPAIR, 128, NK2 * 2 * 2 * R], FP8,
                                    isOutput=False)
    wallA = nc.declare_dram_parameter("wallA", [128, WA_COLS], FP8, isOutput=False)
    wallM = nc.declare_dram_parameter("wallM", [128, WM_COLS], FP8, isOutput=False)
    wallB = nc.declare_dram_parameter("wallB", [128, WB_COLS], FP8, isOutput=False)
    w2w = nc.declare_dram_parameter("w2w", [128, W2_COLS], FP8, isOutput=False)
    cons = nc.declare_dram_parameter("cons", [128, 1], F32, isOutput=False)
    out = nc.declare_dram_parameter("out", [L, R], F32, isOutput=True)

    with TileContext(nc) as tc:
        with (
            tc.tile_pool(name="wpool", bufs=1) as wpool,
            tc.tile_pool(name="tepool", bufs=2) as tepool,
            tc.tile_pool(name="state", bufs=1) as state,
            tc.tile_pool(name="qpool", bufs=3) as qpool,
            tc.tile_pool(name="gpool", bufs=2) as gpool,
            tc.tile_pool(name="appool", bufs=1, space="PSUM") as appool,
            tc.tile_pool(name="hpool", bufs=1, space="PSUM") as hpool,
            tc.tile_pool(name="bpool", bufs=1, space="PSUM") as bpool,
            tc.tile_pool(name="opool", bufs=1, space="PSUM") as opool,
        ):
            # ---- startup DMAs: spread across engine queues so descriptor
            # generation parallelizes; order within each queue by need-time.
            wallM_sb = wpool.tile([128, WM_COLS], FP8, tag="wallM", name="wallM_sb")
            wallB_sb = wpool.tile([128, WB_COLS], FP8, tag="wallB", name="wallB_sb")
            w2_sb = wpool.tile([128, W2_COLS], FP8, tag="w2w", name="w2_sb")
            cons_sb = wpool.tile([128, 1], F32, tag="cons", name="cons_sb")

            def te_dma(tck, pair, c):
                nc.sync.dma_start(
                    tck[:],
                    teT[pair][:, CH_OFF[c] * 4 * R
                              : (CH_OFF[c] + CHUNKS[c]) * 4 * R],
                )

            te0_tiles = []
            wallA_c = []
            for c in range(NCHUNK):
                tck = tepool.tile(
                    [128, CHUNKS[c] * 2 * 2 * R], FP8, tag=f"te{c}",
                    name=f"te_ck{c}",
                )
                te0_tiles.append(tck)
                te_dma(tck, 0, c)
                wa_ck = wpool.tile(
                    [128, CHUNKS[c] * 2 * F], FP8, tag=f"wallA{c}", name=f"wa_ck{c}"
                )
                wallA_c.append(wa_ck)
                nc.scalar.dma_start(
                    wa_ck[:],
                    wallA[:, CH_OFF[c] * 2 * F : (CH_OFF[c] + CHUNKS[c]) * 2 * F],
                )
            nc.gpsimd.dma_start(wallM_sb[:], wallM[:])
            nc.gpsimd.dma_start(wallB_sb[:], wallB[:])
            nc.gpsimd.dma_start(w2_sb[:], w2w[:])
            nc.gpsimd.dma_start(cons_sb[:], cons[:])

            def waT(kk, g):
                # lhsT [128, 2, 128] for DR pair kk, out tile g
                for c in range(NCHUNK - 1, -1, -1):
                    if kk >= CH_OFF[c]:
                        k = kk - CH_OFF[c]
                        v = wallA_c[c][:, k * 2 * F : (k + 1) * 2 * F].rearrange(
                            "p (two f) -> p two f", two=2
                        )
                        return v[:, :, g * 128 : (g + 1) * 128]

            def wbT(kk, g):
                v = wallM_sb[:, O_WB + kk * 2 * F : O_WB + (kk + 1) * 2 * F]
                v = v.rearrange("p (two f) -> p two f", two=2)
                return v[:, :, g * 128 : (g + 1) * 128]

            def seT(kk):
                v = wallM_sb[:, O_SE + kk * 2 * T * NB : O_SE + (kk + 1) * 2 * T * NB]
                return v.rearrange("p (two n) -> p two n", two=2)

            def w1T(kk, g):
                v = wallB_sb[:, kk * 2 * H : (kk + 1) * 2 * H].rearrange(
                    "p (two h) -> p two h", two=2
                )
                return v[:, :, g * 128 : (g + 1) * 128]

            def w2T(kk):
                v = w2_sb[:, kk * 2 * LP : (kk + 1) * 2 * LP]
                v = v.rearrange("p (two l) -> p two l", two=2)
                return v[:, :, 0:L]

            evb_ap = cons_sb[0:L, 0:1]

            # ---- state tiles ----
            M = state.tile([128, NG * R], BF16, tag="M", name="M")
            Mm = state.tile([128, NG * R], BF16, tag="Mm", name="Mm")
            # Y = masked layer-1 psum carry: Y_t = psum_t * (psum_t >= -W_SC);
            # psum_{t+1} = 0.5*Y_t + W_SC*(-W1)@sp  (thresholds double per t).
            # Split in h-halves so each half's preload->W1->compare chain
            # pipelines independently (tile-granular deps).
            Yh = [state.tile([128, 2 * R], BF16, tag=f"Y{h}", name=f"Y{h}")
                  for h in range(2)]
            binc = state.tile([128, NG * T * NB], BF16, tag="binc", name="binc")

            # ap psum: one single-bank tile per g so evictions release banks
            # individually
            ap_ps = [appool.tile([128, 2 * R], F32, tag=f"ap{g}", name=f"ap{g}")
                     for g in range(NG)]
            o_psum = opool.tile([L, R], F32, tag="o", name="o_psum")

            nc.vector.memset(Yh[0][:], 0.0)
            nc.vector.memset(Yh[1][:], 0.0)

            # ---- te DMA + apical matmul emission ----
            def emit_te_dma(pair):
                tiles = []
                for c in range(NCHUNK):
                    tck = tepool.tile(
                        [128, CHUNKS[c] * 2 * 2 * R], FP8, tag=f"te{c}",
                        name=f"te_ck{c}",
                    )
                    tiles.append(tck)
                    te_dma(tck, pair, c)
                return tiles

            def emit_ap_chunk(te_tiles, c):
                for g in range(NG):
                    for k in range(CHUNKS[c]):
                        kk = CH_OFF[c] + k
                        rhs = te_tiles[c][:, k * 4 * R : (k + 1) * 4 * R].rearrange(
                            "p (two n) -> p two n", two=2
                        )
                        nc.tensor.matmul(
                            ap_ps[g][:],
                            lhsT=waT(kk, g),
                            rhs=rhs,
                            start=(kk == 0),
                            stop=(kk == NK2 - 1),
                            perf_mode=DRMODE,
                        )

            def emit_basal():
                bs_psum = bpool.tile([128, NG * T * NB], F32, tag="bs",
                                     name="bs_psum")
                for g in range(NG):
                    for kk in range(NK2):
                        nc.tensor.matmul(
                            bs_psum[:, g * T * NB : (g + 1) * T * NB],
                            lhsT=wbT(kk, g),
                            rhs=seT(kk),
                            start=(kk == 0),
                            stop=(kk == NK2 - 1),
                            perf_mode=DRMODE,
                        )
                # binc (g-major) = cumulative basal contribution to G
                nc.scalar.activation(binc[:], bs_psum[:], ACT_COPY,
                                     scale=PSUM_DESC)

            def binc_bc(t):
                v = binc[:].rearrange("p (g x) -> p g x", g=NG)
                v = v[:, :, t * NB : (t + 1) * NB]
                return v.unsqueeze(3).broadcast_to([128, NG, NB, S])

            def emit_bexp(bx, sub, t):
                # cumulative-basal expansion for timestep t into pair tile
                # half; bx layout is (sub, g, r)
                nc.scalar.activation(
                    bx[:, sub * NG * R : (sub + 1) * NG * R].rearrange(
                        "p (g b s) -> p g b s", g=NG, s=S),
                    binc_bc(t), ACT_COPY,
                )

            # ---- per-pair state math ----
            def emit_P(t0, bx=None):
                # P' = ap_psum * c, one ts per g: releases psum bank g with NO
                # dependency on basal/bexp -- the next pair's matmuls restart
                # immediately. The basal term rides in Mm (see emit_m_chain).
                # For the LAST pair (bx given) bexp is folded in here instead
                # (stt, same cost) so the tail's serial M-chain drops the
                # gpsimd bexp-adds.
                pt = gpool.tile([128, NG * 2 * R], BF16, tag="P", name=f"P{t0}")
                ptv = pt[:].rearrange("p (two gr) -> p two gr", two=2)
                for g in range(NG):
                    src = ap_ps[g][:].rearrange("p (two r) -> p two r", two=2)
                    dst = ptv[:, :, g * R : (g + 1) * R]
                    if bx is None:
                        nc.vector.tensor_scalar(dst, src, PSUM_DESC, None,
                                                OP.mult)
                    else:
                        bxv = bx[:].rearrange("p (two gr) -> p two gr", two=2)
                        nc.vector.scalar_tensor_tensor(
                            dst, src, PSUM_DESC,
                            bxv[:, :, g * R : (g + 1) * R],
                            OP.mult, OP.add,
                        )
                return pt

            def bxs(bx, sub):
                return bx[:, sub * NG * R : (sub + 1) * NG * R]

            def emit_m_chain(pt, sub, t, bx_next, sub_next):
                th0 = float(2 ** (t + 1))
                # M_t = P'_t + Mm  where Mm = q8*M_{t-1} + bexp_t (gpsimd)
                nc.vector.tensor_tensor(
                    M[:], pt[:, sub * NG * R : (sub + 1) * NG * R], Mm[:],
                    OP.add)
                # spike (W1 input) first: it gates the PE
                spq = qpool.tile([128, NG * R], FP8, tag="spq", name=f"spq_{t}")
                nc.vector.tensor_scalar(spq[:], M[:], th0, None, OP.is_gt)
                if t < T - 1:
                    q8 = qpool.tile([128, NG * R], FP8, tag="q8",
                                    name=f"q8_{t}")
                    nc.vector.tensor_scalar(q8[:], M[:], th0, None, OP.is_le)
                    # Mm = q8*M (+ bexp_{t+1} unless it rides in the next
                    # pair's P'), off the DVE queue
                    nc.gpsimd.tensor_tensor(Mm[:], q8[:], M[:], OP.mult)
                    if bx_next is not None:
                        nc.gpsimd.tensor_tensor(
                            Mm[:], Mm[:], bxs(bx_next, sub_next), OP.add)
                return spq

            def emit_w1(t, spq):
                # Per h-half: Act preloads psum with 0.5*Y_{t-1} (exact Copy),
                # W1 DR matmuls (negated wall) accumulate.
                # psum_t = -W_SC/2^t * ml_t.
                hqs = []
                for h in range(2):
                    hq = hpool.tile([128, 2 * R], F32, tag=f"hq{h}",
                                    name=f"hq{h}_{t}")
                    hqs.append(hq)
                    nc.scalar.activation(hq[:], Yh[h][:], ACT_COPY, scale=0.5)
                    for g in (2 * h, 2 * h + 1):
                        for kk in range(NH2):
                            rhs = spq[:, kk * 2 * R : (kk + 1) * 2 * R
                                      ].rearrange("p (two r) -> p two r", two=2)
                            nc.tensor.matmul(
                                hq[:, (g - 2 * h) * R : (g - 2 * h + 1) * R],
                                lhsT=w1T(kk, g),
                                rhs=rhs,
                                start=False,
                                stop=(kk == NH2 - 1),
                                perf_mode=DRMODE,
                                skip_group_check=True,
                            )
                return hqs

            def emit_ml_evict(t, hqs):
                # sp2 = (ml > 2^t) <=> (psum < -W_SC); Y = (sp2==0)*psum is
                # the masked carry (stt allows only one PSUM operand).
                sp2 = qpool.tile([128, NG * R], FP8, tag="sp2", name=f"sp2_{t}")
                for h in range(2):
                    s2h = sp2[:, h * 2 * R : (h + 1) * 2 * R]
                    nc.vector.tensor_scalar(s2h, hqs[h][:], -W_SC, None,
                                            OP.is_lt)
                    if t < T - 1:
                        nc.vector.scalar_tensor_tensor(
                            Yh[h][:], s2h, 0.0, hqs[h][:], OP.is_equal, OP.mult,
                        )
                return sp2

            def emit_w2(t, sp2):
                for kk in range(2):
                    nc.tensor.matmul(
                        o_psum[:],
                        lhsT=w2T(kk),
                        rhs=sp2[:, kk * 2 * R : (kk + 1) * 2 * R].rearrange(
                            "p (two r) -> p two r", two=2
                        ),
                        start=(t == 0 and kk == 0),
                        stop=(t == T - 1 and kk == 1),
                        perf_mode=DRMODE,
                    )

            # ---- prologue: pair-0 apical (DMA-paced), then basal ----
            for c in range(NCHUNK):
                emit_ap_chunk(te0_tiles, c)
            emit_basal()
            bx0 = gpool.tile([128, NG * 2 * R], BF16, tag="bx", name="bx0")
            emit_bexp(bx0, 0, 0)
            emit_bexp(bx0, 1, 1)
            # M_0 = P'_0 + Mm with Mm initialized to bexp_0
            nc.scalar.activation(Mm[:], bx0[:, 0 : NG * R], ACT_COPY)

            # ---- software-pipelined main loop ----
            # Defer each pair's t1 layer-1/2 work into the next pair's
            # emission so PE reaches the next pair's apical matmuls promptly.
            carry = None           # (t1, spqb) of the previous pair
            bx = bx0
            for pair in range(NPAIR):
                t0, t1 = 2 * pair, 2 * pair + 1
                # last pair: bexp folded into P'; its Madds need no gpsimd add
                pt = emit_P(t0, bx if pair == NPAIR - 1 else None)
                spqa = emit_m_chain(pt, 0, t0,
                                    None if pair == NPAIR - 1 else bx, 1)
                if pair + 1 < NPAIR:
                    te_tiles = emit_te_dma(pair + 1)
                    emit_ap_chunk(te_tiles, 0)
                    if carry is not None:
                        tp, spqp = carry
                        hqp = emit_w1(tp, spqp)
                        sp2p = emit_ml_evict(tp, hqp)
                        emit_w2(tp, sp2p)
                    emit_ap_chunk(te_tiles, 1)
                    bxn = gpool.tile([128, NG * 2 * R], BF16, tag="bx",
                                     name=f"bx{pair + 1}")
                    emit_bexp(bxn, 0, t0 + 2)
                    emit_bexp(bxn, 1, t1 + 2)
                    # t5's Mm skips the bexp add: pair 3 carries it in P'
                    spqb = emit_m_chain(pt, 1, t1,
                                        None if pair == NPAIR - 2 else bxn, 0)
                    emit_ap_chunk(te_tiles, 2)
                    hqa = emit_w1(t0, spqa)
                    sp2a = emit_ml_evict(t0, hqa)
                    emit_ap_chunk(te_tiles, 3)
                    emit_ap_chunk(te_tiles, 4)
                    emit_w2(t0, sp2a)
                    bx = bxn
                    carry = (t1, spqb)
                else:
                    if carry is not None:
                        tp, spqp = carry
                        hqp = emit_w1(tp, spqp)
                        sp2p = emit_ml_evict(tp, hqp)
                        emit_w2(tp, sp2p)
                    hqa = emit_w1(t0, spqa)
                    spqb = emit_m_chain(pt, 1, t1, None, None)
                    sp2a = emit_ml_evict(t0, hqa)
                    emit_w2(t0, sp2a)
                    hqb = emit_w1(t1, spqb)
                    sp2b = emit_ml_evict(t1, hqb)
                    emit_w2(t1, sp2b)

            # ---- final eviction: out = o_psum / (T*W_SC) + b2 ----
            out_sb = state.tile([L, R], F32, tag="out_sb", name="out_sb")
            nc.scalar.activation(
                out_sb[:], o_psum[:], ACT_ID,
                bias=evb_ap, scale=1.0 / (T * W_SC),
            )
            nc.sync.dma_start(out[:], out_sb[:])

    return nc


def _swizzle_dr(a, cols):
    """[KD, cols] fp -> fp8 [128, nk*cols]: [p, kt*cols + n] = a[kt*128+p, n]"""
    f8 = ml_dtypes.float8_e4m3
    nk = a.shape[0] // 128
    return np.ascontiguousarray(
        a.reshape(nk, 128, cols).transpose(1, 0, 2).reshape(128, nk * cols).astype(f8)
    )


def prep_in_maps(inputs):
    """Host-side shard + transpose + pad + scale + cumsum + cast."""
    se = np.asarray(inputs["state_embedding"], np.float32)
    te = np.asarray(inputs["tau_embedding"], np.float32)
    Wb = np.asarray(inputs["Wb"], np.float32)
    Wa = np.asarray(inputs["Wa"], np.float32)
    W1 = np.asarray(inputs["W1"], np.float32)
    b1 = np.asarray(inputs["b1"], np.float32)
    W2 = np.asarray(inputs["W2"], np.float32)
    b2 = np.asarray(inputs["b2"], np.float32)
    f8 = ml_dtypes.float8_e4m3

    def padk(a):  # pad feature axis 0 from 3136 to KD
        o = np.zeros((KD,) + a.shape[1:], a.dtype)
        o[: a.shape[0]] = a
        return o

    tsc = (2.0 ** (np.arange(T) - 1)) * EMB_SC   # fold 2^{t-1} into embeddings

    wallA = _swizzle_dr(padk(Wa.T) * W_SC, F)
    wallM_wb = _swizzle_dr(padk(Wb.T) * W_SC, F)

    # W1 wall: 4 k-tiles of -W1^T*512 (negated so the single-scale psum
    # eviction yields ml = mlmask + 2^t*(W1@sp); b1 == 0 in this problem)
    assert not np.any(b1), "nonzero b1 needs an extra constant k-pair"
    wallB = _swizzle_dr(np.ascontiguousarray(W1.T) * -W_SC, H)  # [128, 4*H]

    # W2 wall: [128, kt*LP + l] = W2[l, kt*128+p] * W_SC, 4 k-tiles (2 DR pairs)
    w2w = np.zeros((128, W2_COLS), f8)
    for kt in range(4):
        w2w[:, kt * LP : kt * LP + L] = (W2.T[kt * 128 : (kt + 1) * 128] * W_SC
                                         ).astype(f8)

    cons = np.zeros((128, 1), np.float32)
    cons[:L, 0] = b2

    in_maps = []
    for i in range(N_CORES):
        # teT: [pair, p, kt*2R + sub*R + r] = cumsum_t(te * 2^{t-1} * EMB_SC)
        tei = te[:, i * R : (i + 1) * R, :] * tsc[:, None, None]  # [T, R, DT]
        tei = np.cumsum(tei, axis=0)
        tei = tei.reshape(NPAIR, 2 * R, DT)
        tei_p = np.zeros((NPAIR, 2 * R, KD), np.float32)
        tei_p[:, :, :DT] = tei
        teT = np.ascontiguousarray(
            tei_p.reshape(NPAIR, 2 * R, NK2 * 2, 128)
            .transpose(0, 3, 2, 1)                # [pair, p, ktile, n]
            .reshape(NPAIR, 128, NK2 * 2 * 2 * R)
            .astype(f8)
        )
        # seT region: cumsum over t as well (binc becomes cumulative basal)
        sei = se[:, i * NB : (i + 1) * NB, :] * tsc[:, None, None]  # [T, NB, DS]
        sei = np.cumsum(sei, axis=0)
        seT = padk(np.ascontiguousarray(sei.reshape(T * NB, DS).T))  # [KD, T*NB]
        wallM_i = np.empty((128, WM_COLS), f8)
        wallM_i[:, O_WB : O_WB + NK2 * 2 * F] = wallM_wb
        wallM_i[:, O_SE :] = _swizzle_dr(seT, T * NB)
        in_maps.append(dict(teT=teT, wallA=wallA, wallM=wallM_i, wallB=wallB,
                            w2w=w2w, cons=cons))
    return in_maps


def assemble_out(core_outs):
    """[N_CORES][L, R] -> [B, L, S]"""
    full = np.stack([np.asarray(o, np.float32) for o in core_outs], axis=0)
    full = full.reshape(N_CORES, L, NB, S).transpose(0, 2, 1, 3)
    return np.ascontiguousarray(full.reshape(B, L, S))


_NC_CACHE = {}


def get_nc():
    if "nc" not in _NC_CACHE:
        last = None
        for _ in range(6):
            try:
                _NC_CACHE["nc"] = build_nc()
                break
            except Exception as e:  # rare scheduler-order race-detector trip
                last = e
        else:
            raise last
    return _NC_CACHE["nc"]


def run_sharded(in_maps, trace=False, **kw):
    nc = get_nc()
    if not getattr(nc, "_waits_split", False):
        _split_excess_waits(nc)
        nc._waits_split = True
    res = run_bass_kernel_spmd(
        nc, in_maps, core_ids=list(range(N_CORES)), trace=trace, **kw
    )
    return res


def kernel(**inputs):
    in_maps = prep_in_maps(inputs)
    res = run_sharded(in_maps)
    return assemble_out([res.results[i]["out"] for i in range(N_CORES)])
